# revision 12
# baseline (speedup 1.0000x reference)
"""Additive attention (Bahdanau) Trainium2 kernel, SPMD over 8 NeuronCores.

Math per batch b (see reference):
    q = queries[b] @ Wq                  [Q=128, H=256]
    k = keys[b]    @ Wk                  [K=1024, H=256]
    scores[i,j] = sum_h wv[h] * tanh(q[i,h] + k[j,h])
    attn = masked_softmax(scores, valid_len[b])
    out[b] = attn @ values[b]            [Q, V=512]

Sharding: data-parallel, batch c -> core c.  No collectives.

Device pipeline (per core), all h-on-partitions ("Layout B"):
  - qh = (queries @ Wq).T   [2][128h, 128q]   (PE, contraction d on partitions,
    host pre-transposes queries/keys so d is already the leading dim)
  - kh = (keys @ Wk).T      [2][128h, 1024k]
  - S[h, q, k] = kh[h,k] + qh[h,q]  via DVE tensor_scalar_add (per-partition
    scalar = qh column; bf16 SBUF -> 4x mode)
  - feat = tanh(S)          (ACT, big packed ops)
  - scores row [1, 512] = wv.T @ feat  (M=1 matmuls, accumulated over the two
    h-tiles; rows land on PSUM partitions {0,32,64,96} - slab layout)
  - slab -> scores_sb [128q, 1024k] via one strided DMA per 4-q group
  - exp = Exp(scores)  (no max subtraction: |scores| <= sum|wv| ~ 13, exp is
    safely inside fp32 range, softmax is shift-invariant)
  - mask: host zeroes rows k >= valid_len of values_ext; its last column is
    1[k < valid_len], so attn @ values_ext also yields the softmax denominator.
  - expT tiles via PE transpose; out = (expT.T @ values_ext) * recip(l)
"""

import os
import sys

import numpy as np

for _p in ("/opt/trn_rl_repo", "/root/.axon_site/_ro/trn_rl_repo"):
    if os.path.isdir(_p) and _p not in sys.path:
        sys.path.insert(0, _p)

os.environ.setdefault("MYCRO_LOCAL_CACHE", "1")

import ml_dtypes  # noqa: E402
from contextlib import ExitStack  # noqa: E402

import concourse.bass as bass  # noqa: E402
import concourse.tile as tile  # noqa: E402
from concourse import bacc, mybir  # noqa: E402
from concourse.bass_utils import run_bass_kernel_spmd  # noqa: E402
from concourse.masks import make_identity  # noqa: E402

BF16 = mybir.dt.bfloat16
F32 = mybir.dt.float32
NP_BF16 = ml_dtypes.bfloat16

B, Q, K, D, H, V = 8, 128, 1024, 512, 256, 512
DC = D // 128   # 4 contraction tiles for the projections
HT = H // 128   # 2 h-tiles
KTILES = K // 128
QG = 4          # q rows per score-slab group
VE = V + 1      # values extended with a ones column (softmax denominator)


def _build_graph():
    nc = bacc.Bacc(
        "TRN2",
        target_bir_lowering=False,
        debug=False,
        num_devices=8,
    )

    qT = nc.dram_tensor("qT", [D, Q], BF16, kind="ExternalInput")
    kT = nc.dram_tensor("kT", [D, K], BF16, kind="ExternalInput")
    vext = nc.dram_tensor("vext", [K, VE], BF16, kind="ExternalInput")
    wq = nc.dram_tensor("wq", [D, H], BF16, kind="ExternalInput")
    wk = nc.dram_tensor("wk", [D, H], BF16, kind="ExternalInput")
    wv2 = nc.dram_tensor("wv2", [128, HT], BF16, kind="ExternalInput")
    out = nc.dram_tensor("out", [Q, V], F32, kind="ExternalOutput")

    with tile.TileContext(nc) as tc, ExitStack() as ctx:
        singles = ctx.enter_context(tc.tile_pool(name="singles", bufs=1))
        work = ctx.enter_context(tc.tile_pool(name="work", bufs=2))
        psum = ctx.enter_context(tc.tile_pool(name="psum", bufs=1, space="PSUM"))

        # ---- load inputs ------------------------------------------------
        wq_sb = singles.tile([128, DC, H], BF16)
        nc.gpsimd.dma_start(wq_sb[:], wq.ap().rearrange("(c p) h -> p c h", p=128))
        wk_sb = singles.tile([128, DC, H], BF16)
        nc.gpsimd.dma_start(wk_sb[:], wk.ap().rearrange("(c p) h -> p c h", p=128))
        qt_sb = singles.tile([128, DC, Q], BF16)
        nc.gpsimd.dma_start(qt_sb[:], qT.ap().rearrange("(c p) q -> p c q", p=128))
        kt_sb = singles.tile([128, DC, K], BF16)
        nc.gpsimd.dma_start(kt_sb[:], kT.ap().rearrange("(c p) k -> p c k", p=128))
        v_sb = singles.tile([128, KTILES, VE], BF16)
        nc.gpsimd.dma_start(v_sb[:], vext.ap().rearrange("(c p) v -> p c v", p=128))
        wv_sb = singles.tile([128, HT], BF16)
        nc.gpsimd.dma_start(wv_sb[:], wv2.ap())
        ident = singles.tile([128, 128], BF16)
        make_identity(nc, ident[:])

        # ---- projections: qh [128, HT, Q], kh [128, HT, K] (bf16) -------
        qh_sb = singles.tile([128, HT, Q], F32)
        kh_sb = singles.tile([128, HT, K], BF16)
        for ht in range(HT):
            ps = psum.tile([128, 512], F32, tag="misc", bufs=2)
            for dc in range(DC):
                nc.tensor.matmul(
                    ps[:, :Q],
                    lhsT=wq_sb[:, dc, ht * 128 : (ht + 1) * 128],
                    rhs=qt_sb[:, dc, :],
                    start=(dc == 0),
                    stop=(dc == DC - 1),
                )
            nc.vector.tensor_copy(qh_sb[:, ht, :], ps[:, :Q])
        for ht in range(HT):
            for kc in range(K // 512):
                ps = psum.tile([128, 512], F32, tag="misc", bufs=2)
                for dc in range(DC):
                    nc.tensor.matmul(
                        ps[:],
                        lhsT=wk_sb[:, dc, ht * 128 : (ht + 1) * 128],
                        rhs=kt_sb[:, dc, kc * 512 : (kc + 1) * 512],
                        start=(dc == 0),
                        stop=(dc == DC - 1),
                    )
                nc.vector.tensor_copy(kh_sb[:, ht, kc * 512 : (kc + 1) * 512], ps[:])

        # ---- scores: for each 4-q group build S, tanh, reduce over h ----
        scores_sb = singles.tile([128, K], F32)
        slabs = [
            psum.tile([128, K], F32, tag=f"slab{i}", name=f"slab{i}")
            for i in range(2)
        ]
        for s in slabs:
            # initialize the never-written partitions so the full-tile drain
            # copy below reads defined memory
            nc.vector.memset(s[:], 0.0)
        for g in range(Q // QG):
            S0 = work.tile([128, QG, K], BF16, tag="S0")
            S1 = work.tile([128, QG, K], BF16, tag="S1")
            for j in range(QG):
                qi = g * QG + j
                nc.vector.tensor_scalar_add(
                    S0[:, j, :], kh_sb[:, 0, :], qh_sb[:, 0, qi : qi + 1]
                )
                nc.vector.tensor_scalar_add(
                    S1[:, j, :], kh_sb[:, 1, :], qh_sb[:, 1, qi : qi + 1]
                )
            F0 = work.tile([128, QG, K], BF16, tag="F0")
            F1 = work.tile([128, QG, K], BF16, tag="F1")
            nc.scalar.activation(F0[:], S0[:], mybir.ActivationFunctionType.Tanh)
            nc.scalar.activation(F1[:], S1[:], mybir.ActivationFunctionType.Tanh)

            # M=1 matmuls: row (j, c) -> psum partition 32*j, bank c
            slab = slabs[g % 2]
            for j in range(QG):
                for c in range(K // 512):
                    out_ap = slab[32 * j : 32 * j + 1, c * 512 : (c + 1) * 512]
                    nc.tensor.matmul(
                        out_ap,
                        lhsT=wv_sb[:, 0:1],
                        rhs=F0[:, j, c * 512 : (c + 1) * 512],
                        start=True,
                        stop=False,
                        tile_position=(0, 32 * j),
                    )
                    nc.tensor.matmul(
                        out_ap,
                        lhsT=wv_sb[:, 1:2],
                        rhs=F1[:, j, c * 512 : (c + 1) * 512],
                        start=False,
                        stop=True,
                        tile_position=(0, 32 * j),
                    )
            # drain slab -> SBUF staging (partition-preserving engine copy;
            # DMA cannot read PSUM), then compact partitions 0/32/64/96 ->
            # scores rows via SBUF->SBUF strided DMA.
            stage = work.tile([128, K], F32, tag="stage")
            nc.vector.tensor_copy(stage[:], slab[:])
            nc.gpsimd.dma_start(
                scores_sb[g * QG : (g + 1) * QG, :], stage[0:128:32, :]
            )

        # ---- softmax (no max subtraction) + AV --------------------------
        exp_sb = singles.tile([128, K], BF16)
        nc.scalar.activation(
            exp_sb[:], scores_sb[:], mybir.ActivationFunctionType.Exp
        )

        expT_sb = singles.tile([128, KTILES, 128], BF16)
        for kt in range(KTILES):
            pt = psum.tile([128, 128], BF16, tag="misc", bufs=2)
            nc.tensor.transpose(
                pt[:], exp_sb[:, kt * 128 : (kt + 1) * 128], ident[:]
            )
            nc.vector.tensor_copy(expT_sb[:, kt, :], pt[:])

        av = psum.tile([128, 1024], F32, tag="av", bufs=1)
        for kt in range(KTILES):
            nc.tensor.matmul(
                av[:, 0:V],
                lhsT=expT_sb[:, kt, :],
                rhs=v_sb[:, kt, 0:V],
                start=(kt == 0),
                stop=(kt == KTILES - 1),
            )
            nc.tensor.matmul(
                av[:, 512:513],
                lhsT=expT_sb[:, kt, :],
                rhs=v_sb[:, kt, V : V + 1],
                start=(kt == 0),
                stop=(kt == KTILES - 1),
            )

        rl = singles.tile([128, 1], F32)
        nc.vector.reciprocal(rl[:], av[:, 512:513])
        out_sb = singles.tile([128, V], F32)
        nc.vector.tensor_scalar_mul(out_sb[:], av[:, 0:V], rl[:])
        nc.gpsimd.dma_start(out.ap(), out_sb[:])

    nc.compile()
    return nc


_CACHE = {}


def _install_profile_shim():
    """Provide antenv.axon_hooks (absent in this image) so
    run_bass_kernel_spmd(trace=True) can capture NTFF profiles through
    libaxon_pjrt.so, mirroring trn_agent_boot's bootstrap."""
    import types

    if "antenv.axon_hooks" not in sys.modules:
        mod = types.ModuleType("antenv.axon_hooks")
        state = {}
        mod.set_axon_ntff_profile_hook = lambda h: state.__setitem__("h", h)
        mod.get_axon_ntff_profile_hook = lambda: state.get("h")
        sys.modules["antenv.axon_hooks"] = mod
        import antenv

        antenv.axon_hooks = mod
        if "/root/.axon_site" not in sys.path:
            sys.path.insert(0, "/root/.axon_site")
        from trn_agent_boot.trn_boot import _ntff_profile_via_ctypes

        hook = _ntff_profile_via_ctypes("/opt/axon/libaxon_pjrt.so")
        mod.set_axon_ntff_profile_hook(hook)

        import concourse.bass_utils as bu

        orig_upload = bu.upload_artifacts

        def _safe_upload(tmpdir):
            try:
                return orig_upload(tmpdir)
            except Exception:
                return f"local:{tmpdir}"

        bu.upload_artifacts = _safe_upload


def _get_graph():
    if "nc" not in _CACHE:
        _CACHE["nc"] = _build_graph()
    return _CACHE["nc"]


def _make_in_maps(queries, keys, values, valid_lens):
    in_maps = []
    for c in range(B):
        vl = int(valid_lens[c])
        vext = np.zeros((K, VE), dtype=np.float32)
        vext[:vl, :V] = values[c, :vl]
        vext[:vl, V] = 1.0
        in_maps.append(
            {
                "qT": np.ascontiguousarray(queries[c].T).astype(NP_BF16),
                "kT": np.ascontiguousarray(keys[c].T).astype(NP_BF16),
                "vext": vext.astype(NP_BF16),
                "wq": _CACHE["wq_bf"],
                "wk": _CACHE["wk_bf"],
                "wv2": _CACHE["wv2_bf"],
            }
        )
    return in_maps


def kernel(
    queries, keys, values, valid_lens, Wq, Wk, wv, _profile=False, **_unused
):
    queries = np.asarray(queries, dtype=np.float32)
    keys = np.asarray(keys, dtype=np.float32)
    values = np.asarray(values, dtype=np.float32)
    valid_lens = np.asarray(valid_lens)
    _CACHE["wq_bf"] = np.asarray(Wq, np.float32).astype(NP_BF16)
    _CACHE["wk_bf"] = np.asarray(Wk, np.float32).astype(NP_BF16)
    _CACHE["wv2_bf"] = (
        np.asarray(wv, np.float32).reshape(HT, 128).T.copy().astype(NP_BF16)
    )

    nc = _get_graph()
    in_maps = _make_in_maps(queries, keys, values, valid_lens)
    kwargs = {}
    if _profile:
        _install_profile_shim()
        tdir = "/root/problem/trace_out"
        os.makedirs(tdir, exist_ok=True)
        kwargs["tmpdir"] = tdir
    res = run_bass_kernel_spmd(
        nc, in_maps, core_ids=list(range(B)), trace=_profile, **kwargs
    )
    out = np.stack([res.results[c]["out"] for c in range(B)]).astype(np.float32)
    if _profile:
        _CACHE["last_result"] = res
    return out


# revision 13
# speedup vs baseline: 1.1124x; 1.1124x over previous
"""Additive attention (Bahdanau) Trainium2 kernel, SPMD over 8 NeuronCores.

Math per batch b (see reference):
    q = queries[b] @ Wq                  [Q=128, H=256]
    k = keys[b]    @ Wk                  [K=1024, H=256]
    scores[i,j] = sum_h wv[h] * tanh(q[i,h] + k[j,h])
    attn = masked_softmax(scores, valid_len[b])
    out[b] = attn @ values[b]            [Q, V=512]

Sharding: sequence-parallel q-striping. Each core takes 16 q-rows of EVERY
batch and only the valid k-range of each batch (rounded up to 128). Per-core
work = sum_b 16*ceil(vl_b/128)*128 columns -- perfectly balanced for any
valid_lens, no collectives (softmax is per-q-row and stays core-local).

Device pipeline (per core), h-on-partitions layout:
  - qh = (queries_rows @ Wq).T  [2][128h, 128 q-slots]  (slot 16*b+r = batch
    b, row 16*core+r; host pre-transposes queries/keys so the contraction
    dim d is leading)
  - kh = (keys @ Wk).T          [2][128h, KEXT]  (KEXT = sum_b KW_b)
  - per q-slot: S = kh[:, group cols] + qh[:, slot]  (DVE tensor_scalar_add,
    bf16 4x mode), tanh in place (ACT), then scores row = wv.T @ feat via
    M=1 matmuls accumulated over the two h-tiles into PSUM slab rows at
    partitions {0,32,64,96}
  - slab -> stage (DVE copy) -> strided SBUF DMA -> scores_sb[q-slots, cols]
  - scores_sb pre-filled with -60 so cross-batch cells exp to ~0
  - exp (no max subtraction: |scores| <= sum|wv| ~ 13, safely in fp32 range)
  - mask is baked into values_ext: rows k >= vl zeroed, last column is
    1[k < vl], so attn @ values_ext also yields the softmax denominator
  - expT tiles via PE transpose; out = (expT.T @ values_ext) * recip(l)
"""

import os
import sys

import numpy as np

for _p in ("/opt/trn_rl_repo", "/root/.axon_site/_ro/trn_rl_repo"):
    if os.path.isdir(_p) and _p not in sys.path:
        sys.path.insert(0, _p)

os.environ.setdefault("MYCRO_LOCAL_CACHE", "1")

import ml_dtypes  # noqa: E402
from contextlib import ExitStack  # noqa: E402

import concourse.bass as bass  # noqa: E402
import concourse.tile as tile  # noqa: E402
from concourse import bacc, mybir  # noqa: E402
from concourse.bass_utils import run_bass_kernel_spmd  # noqa: E402
from concourse.masks import make_identity  # noqa: E402

BF16 = mybir.dt.bfloat16
F32 = mybir.dt.float32
NP_BF16 = ml_dtypes.bfloat16

B, Q, K, D, H, V = 8, 128, 1024, 512, 256, 512
DC = D // 128   # 4 contraction tiles for the projections
HT = H // 128   # 2 h-tiles
QPC = Q // B    # 16 q-rows per (batch, core)
VE = V + 1      # values extended with a ones column (softmax denominator)
NEG = -60.0     # filler for never-written score cells; exp(-60) ~ 9e-27


def _kw_template(valid_lens):
    kw = [max(128, int(-(-int(v) // 128) * 128)) for v in valid_lens]
    koff = np.concatenate([[0], np.cumsum(kw)]).astype(int)
    return kw, koff, int(koff[-1])


def _build_graph(valid_lens):
    kws, koff, KEXT = _kw_template(valid_lens)
    nc = bacc.Bacc(
        "TRN2",
        target_bir_lowering=False,
        debug=False,
        num_devices=8,
    )

    qT = nc.dram_tensor("qT", [D, Q], BF16, kind="ExternalInput")
    kT = nc.dram_tensor("kT", [D, KEXT], BF16, kind="ExternalInput")
    vext = nc.dram_tensor("vext", [KEXT, VE], BF16, kind="ExternalInput")
    wq = nc.dram_tensor("wq", [D, H], BF16, kind="ExternalInput")
    wk = nc.dram_tensor("wk", [D, H], BF16, kind="ExternalInput")
    wv2 = nc.dram_tensor("wv2", [128, HT], BF16, kind="ExternalInput")
    out = nc.dram_tensor("out", [Q, V], F32, kind="ExternalOutput")

    KT = KEXT // 128  # 128-col k-tiles (template is 128-aligned)

    with tile.TileContext(nc) as tc, ExitStack() as ctx:
        singles = ctx.enter_context(tc.tile_pool(name="singles", bufs=1))
        work = ctx.enter_context(tc.tile_pool(name="work", bufs=2))
        psum = ctx.enter_context(tc.tile_pool(name="psum", bufs=1, space="PSUM"))

        # ---- load inputs ------------------------------------------------
        wq_sb = singles.tile([128, DC, H], BF16)
        nc.gpsimd.dma_start(wq_sb[:], wq.ap().rearrange("(c p) h -> p c h", p=128))
        wk_sb = singles.tile([128, DC, H], BF16)
        nc.gpsimd.dma_start(wk_sb[:], wk.ap().rearrange("(c p) h -> p c h", p=128))
        qt_sb = singles.tile([128, DC, Q], BF16)
        nc.gpsimd.dma_start(qt_sb[:], qT.ap().rearrange("(c p) q -> p c q", p=128))
        kt_sb = singles.tile([128, DC, KEXT], BF16)
        nc.gpsimd.dma_start(kt_sb[:], kT.ap().rearrange("(c p) k -> p c k", p=128))
        v_sb = singles.tile([128, KT, VE], BF16)
        nc.gpsimd.dma_start(v_sb[:], vext.ap().rearrange("(c p) v -> p c v", p=128))
        wv_sb = singles.tile([128, HT], BF16)
        nc.gpsimd.dma_start(wv_sb[:], wv2.ap())
        ident = singles.tile([128, 128], BF16)
        make_identity(nc, ident[:])

        # ---- projections: qh [128, HT, Q] f32, kh [128, HT, KEXT] bf16 --
        qh_sb = singles.tile([128, HT, Q], F32)
        kh_sb = singles.tile([128, HT, KEXT], BF16)
        for ht in range(HT):
            ps = psum.tile([128, 512], F32, tag="misc", bufs=2)
            for dc in range(DC):
                nc.tensor.matmul(
                    ps[:, :Q],
                    lhsT=wq_sb[:, dc, ht * 128 : (ht + 1) * 128],
                    rhs=qt_sb[:, dc, :],
                    start=(dc == 0),
                    stop=(dc == DC - 1),
                )
            nc.vector.tensor_copy(qh_sb[:, ht, :], ps[:, :Q])
        for ht in range(HT):
            for kc in range((KEXT + 511) // 512):
                cw = min(512, KEXT - kc * 512)
                ps = psum.tile([128, 512], F32, tag="misc", bufs=2)
                for dc in range(DC):
                    nc.tensor.matmul(
                        ps[:, :cw],
                        lhsT=wk_sb[:, dc, ht * 128 : (ht + 1) * 128],
                        rhs=kt_sb[:, dc, kc * 512 : kc * 512 + cw],
                        start=(dc == 0),
                        stop=(dc == DC - 1),
                    )
                nc.vector.tensor_copy(
                    kh_sb[:, ht, kc * 512 : kc * 512 + cw], ps[:, :cw]
                )

        # ---- scores + per-group epilogue --------------------------------
        scores_sb = singles.tile([128, KEXT], F32)
        nc.vector.memset(scores_sb[:], NEG)
        exp_sb = singles.tile([128, KEXT], BF16)
        expT_sb = singles.tile([128, KT, 128], BF16)
        av = psum.tile([128, 1024], F32, tag="av", bufs=1)
        slabs = [
            psum.tile([128, 1024], F32, tag=f"slab{i}", name=f"slab{i}")
            for i in range(2)
        ]
        for s in slabs:
            nc.vector.memset(s[:], 0.0)

        slab_i = 0
        kt_done = 0
        for g in range(B):
            kw, off = kws[g], int(koff[g])
            nchunk = (kw + 511) // 512
            for jg in range(QPC // 8):  # two 8-row chunks per group
                s0 = work.tile([128, 8, kw], BF16, tag="S0", name="s0")
                s1 = work.tile([128, 8, kw], BF16, tag="S1", name="s1")
                for j in range(8):
                    slot = g * QPC + jg * 8 + j
                    nc.vector.tensor_scalar_add(
                        s0[:, j, :],
                        kh_sb[:, 0, off : off + kw],
                        qh_sb[:, 0, slot : slot + 1],
                    )
                    nc.vector.tensor_scalar_add(
                        s1[:, j, :],
                        kh_sb[:, 1, off : off + kw],
                        qh_sb[:, 1, slot : slot + 1],
                    )
                # tanh in place
                nc.scalar.activation(
                    s0[:], s0[:], mybir.ActivationFunctionType.Tanh
                )
                nc.scalar.activation(
                    s1[:], s1[:], mybir.ActivationFunctionType.Tanh
                )
                for half in range(2):  # 4-slot slabs
                    slab = slabs[slab_i % 2]
                    slab_i += 1
                    for j4 in range(4):
                        j = half * 4 + j4
                        for c in range(nchunk):
                            cw = min(512, kw - c * 512)
                            out_ap = slab[
                                32 * j4 : 32 * j4 + 1, c * 512 : c * 512 + cw
                            ]
                            nc.tensor.matmul(
                                out_ap,
                                lhsT=wv_sb[:, 0:1],
                                rhs=s0[:, j, c * 512 : c * 512 + cw],
                                start=True,
                                stop=False,
                                tile_position=(0, 32 * j4),
                            )
                            nc.tensor.matmul(
                                out_ap,
                                lhsT=wv_sb[:, 1:2],
                                rhs=s1[:, j, c * 512 : c * 512 + cw],
                                start=False,
                                stop=True,
                                tile_position=(0, 32 * j4),
                            )
                    stage = work.tile([128, 1024], F32, tag="stage", name="stage")
                    nc.vector.tensor_copy(stage[:, :kw], slab[:, :kw])
                    row0 = g * QPC + jg * 8 + half * 4
                    nc.gpsimd.dma_start(
                        scores_sb[row0 : row0 + 4, off : off + kw],
                        stage[0:128:32, :kw],
                    )

            # group epilogue: exp over this group's columns, transpose its
            # k-tiles, accumulate AV (runs while later groups compute)
            nc.scalar.activation(
                exp_sb[:, off : off + kw],
                scores_sb[:, off : off + kw],
                mybir.ActivationFunctionType.Exp,
            )
            for t in range(kw // 128):
                kt = off // 128 + t
                pt = psum.tile([128, 128], BF16, tag="misc", bufs=2)
                nc.tensor.transpose(
                    pt[:], exp_sb[:, kt * 128 : (kt + 1) * 128], ident[:]
                )
                nc.vector.tensor_copy(expT_sb[:, kt, :], pt[:])
                nc.tensor.matmul(
                    av[:, 0:V],
                    lhsT=expT_sb[:, kt, :],
                    rhs=v_sb[:, kt, 0:V],
                    start=(kt_done == 0),
                    stop=(kt_done == KT - 1),
                )
                nc.tensor.matmul(
                    av[:, 512:513],
                    lhsT=expT_sb[:, kt, :],
                    rhs=v_sb[:, kt, V : V + 1],
                    start=(kt_done == 0),
                    stop=(kt_done == KT - 1),
                )
                kt_done += 1

        rl = singles.tile([128, 1], F32)
        nc.vector.reciprocal(rl[:], av[:, 512:513])
        out_sb = singles.tile([128, V], F32)
        nc.vector.tensor_scalar_mul(out_sb[:], av[:, 0:V], rl[:])
        nc.gpsimd.dma_start(out.ap(), out_sb[:])

    nc.compile()
    return nc


_CACHE = {}


def _install_profile_shim():
    """Provide antenv.axon_hooks (absent in this image) so
    run_bass_kernel_spmd(trace=True) can capture NTFF profiles through
    libaxon_pjrt.so, mirroring trn_agent_boot's bootstrap."""
    import types

    if "antenv.axon_hooks" not in sys.modules:
        mod = types.ModuleType("antenv.axon_hooks")
        state = {}
        mod.set_axon_ntff_profile_hook = lambda h: state.__setitem__("h", h)
        mod.get_axon_ntff_profile_hook = lambda: state.get("h")
        sys.modules["antenv.axon_hooks"] = mod
        import antenv

        antenv.axon_hooks = mod
        if "/root/.axon_site" not in sys.path:
            sys.path.insert(0, "/root/.axon_site")
        from trn_agent_boot.trn_boot import _ntff_profile_via_ctypes

        hook = _ntff_profile_via_ctypes("/opt/axon/libaxon_pjrt.so")
        mod.set_axon_ntff_profile_hook(hook)

        import concourse.bass_utils as bu

        orig_upload = bu.upload_artifacts

        def _safe_upload(tmpdir):
            try:
                return orig_upload(tmpdir)
            except Exception:
                return f"local:{tmpdir}"

        bu.upload_artifacts = _safe_upload


def _get_graph(valid_lens):
    key = tuple(int(v) for v in valid_lens)
    if _CACHE.get("key") != key:
        _CACHE["nc"] = _build_graph(valid_lens)
        _CACHE["key"] = key
    return _CACHE["nc"]


def _make_in_maps(queries, keys, values, valid_lens):
    kws, koff, KEXT = _kw_template(valid_lens)
    kT = np.zeros((D, KEXT), dtype=np.float32)
    vext = np.zeros((KEXT, VE), dtype=np.float32)
    for b in range(B):
        vl = int(valid_lens[b])
        kw, off = kws[b], int(koff[b])
        kT[:, off : off + kw] = keys[b, :kw].T
        vext[off : off + vl, :V] = values[b, :vl]
        vext[off : off + vl, V] = 1.0
    kT_bf = kT.astype(NP_BF16)
    vext_bf = vext.astype(NP_BF16)
    in_maps = []
    for c in range(B):
        qrows = np.concatenate(
            [queries[b, c * QPC : (c + 1) * QPC] for b in range(B)], axis=0
        )  # [128, D]; slot 16*b + r = (batch b, row 16*c + r)
        in_maps.append(
            {
                "qT": np.ascontiguousarray(qrows.T).astype(NP_BF16),
                "kT": kT_bf,
                "vext": vext_bf,
                "wq": _CACHE["wq_bf"],
                "wk": _CACHE["wk_bf"],
                "wv2": _CACHE["wv2_bf"],
            }
        )
    return in_maps


def kernel(
    queries, keys, values, valid_lens, Wq, Wk, wv, _profile=False, **_unused
):
    queries = np.asarray(queries, dtype=np.float32)
    keys = np.asarray(keys, dtype=np.float32)
    values = np.asarray(values, dtype=np.float32)
    valid_lens = np.asarray(valid_lens)
    _CACHE["wq_bf"] = np.asarray(Wq, np.float32).astype(NP_BF16)
    _CACHE["wk_bf"] = np.asarray(Wk, np.float32).astype(NP_BF16)
    _CACHE["wv2_bf"] = (
        np.asarray(wv, np.float32).reshape(HT, 128).T.copy().astype(NP_BF16)
    )

    nc = _get_graph(valid_lens)
    in_maps = _make_in_maps(queries, keys, values, valid_lens)
    kwargs = {}
    if _profile:
        _install_profile_shim()
        tdir = "/root/problem/trace_out"
        os.makedirs(tdir, exist_ok=True)
        kwargs["tmpdir"] = tdir
    res = run_bass_kernel_spmd(
        nc, in_maps, core_ids=list(range(B)), trace=_profile, **kwargs
    )
    out = np.zeros((B, Q, V), dtype=np.float32)
    for c in range(B):
        oc = np.asarray(res.results[c]["out"], dtype=np.float32)
        for b in range(B):
            out[b, c * QPC : (c + 1) * QPC] = oc[b * QPC : (b + 1) * QPC]
    if _profile:
        _CACHE["last_result"] = res
    return out


# revision 17
# speedup vs baseline: 1.3226x; 1.1890x over previous
"""Additive attention (Bahdanau) Trainium2 kernel, SPMD over 8 NeuronCores.

Math per batch b (see reference):
    q = queries[b] @ Wq                  [Q=128, H=256]
    k = keys[b]    @ Wk                  [K=1024, H=256]
    scores[i,j] = sum_h wv[h] * tanh(q[i,h] + k[j,h])
    attn = masked_softmax(scores, valid_len[b])
    out[b] = attn @ values[b]            [Q, V=512]

Sharding: sequence-parallel q-striping. Each core takes 16 q-rows of EVERY
batch and only the valid k-range of each batch (rounded up to 128). Per-core
work = sum_b 16*ceil(vl_b/128)*128 columns -- perfectly balanced for any
valid_lens, no collectives (softmax is per-q-row and stays core-local).

Device pipeline (per core), h-on-partitions layout:
  - qh = (queries_rows @ Wq).T  [2][128h, 128 q-slots]  (slot 16*b+r = batch
    b, row 16*core+r; host pre-transposes queries/keys so the contraction
    dim d is leading)
  - kh = (keys @ Wk).T          [2][128h, KEXT]  (KEXT = sum_b KW_b)
  - per q-slot: S = kh[:, group cols] + qh[:, slot]  (DVE tensor_scalar_add,
    bf16 4x mode), tanh in place (ACT), then scores row = wv.T @ feat via
    M=1 matmuls accumulated over the two h-tiles into PSUM slab rows at
    partitions {0,32,64,96}
  - slab -> stage (DVE copy) -> strided SBUF DMA -> scores_sb[q-slots, cols]
  - scores_sb pre-filled with -60 so cross-batch cells exp to ~0
  - exp (no max subtraction: |scores| <= sum|wv| ~ 13, safely in fp32 range)
  - mask is baked into values_ext: rows k >= vl zeroed, last column is
    1[k < vl], so attn @ values_ext also yields the softmax denominator
  - expT tiles via PE transpose; out = (expT.T @ values_ext) * recip(l)
"""

import os
import sys

import numpy as np

for _p in ("/opt/trn_rl_repo", "/root/.axon_site/_ro/trn_rl_repo"):
    if os.path.isdir(_p) and _p not in sys.path:
        sys.path.insert(0, _p)

os.environ.setdefault("MYCRO_LOCAL_CACHE", "1")

import ml_dtypes  # noqa: E402
from contextlib import ExitStack  # noqa: E402

import concourse.bass as bass  # noqa: E402
import concourse.tile as tile  # noqa: E402
from concourse import bacc, mybir  # noqa: E402
from concourse.bass_utils import run_bass_kernel_spmd  # noqa: E402
from concourse.masks import make_identity  # noqa: E402

BF16 = mybir.dt.bfloat16
F32 = mybir.dt.float32
NP_BF16 = ml_dtypes.bfloat16

B, Q, K, D, H, V = 8, 128, 1024, 512, 256, 512
DC = D // 128   # 4 contraction tiles for the projections
HT = H // 128   # 2 h-tiles
QPC = Q // B    # 16 q-rows per (batch, core)
VE = V + 1      # values extended with a ones column (softmax denominator)
NEG = -60.0     # filler for never-written score cells; exp(-60) ~ 9e-27


def _kw_template(valid_lens):
    kw = [max(128, int(-(-int(v) // 128) * 128)) for v in valid_lens]
    koff = np.concatenate([[0], np.cumsum(kw)]).astype(int)
    return kw, koff, int(koff[-1])


def _build_graph(valid_lens):
    kws, koff, KEXT = _kw_template(valid_lens)
    nc = bacc.Bacc(
        "TRN2",
        target_bir_lowering=False,
        debug=False,
        num_devices=8,
    )

    qT = nc.dram_tensor("qT", [D, Q], BF16, kind="ExternalInput")
    kT = nc.dram_tensor("kT", [D, KEXT], BF16, kind="ExternalInput")
    vext = nc.dram_tensor("vext", [KEXT, VE], BF16, kind="ExternalInput")
    wq = nc.dram_tensor("wq", [D, H], BF16, kind="ExternalInput")
    wk = nc.dram_tensor("wk", [D, H], BF16, kind="ExternalInput")
    wv2 = nc.dram_tensor("wv2", [128, HT], BF16, kind="ExternalInput")
    out = nc.dram_tensor("out", [Q, V], F32, kind="ExternalOutput")

    KT = KEXT // 128  # 128-col k-tiles (template is 128-aligned)

    with tile.TileContext(nc) as tc, ExitStack() as ctx:
        singles = ctx.enter_context(tc.tile_pool(name="singles", bufs=1))
        work = ctx.enter_context(tc.tile_pool(name="work", bufs=2))
        psum = ctx.enter_context(tc.tile_pool(name="psum", bufs=1, space="PSUM"))

        # ---- load inputs ------------------------------------------------
        # tiny warmup activation so the ~2.7us ACT table load overlaps DMAs
        warm = singles.tile([1, 2], F32)
        nc.vector.memset(warm[:], 0.0)
        nc.scalar.activation(warm[:], warm[:], mybir.ActivationFunctionType.Tanh)

        wq_sb = singles.tile([128, DC, H], BF16)
        nc.sync.dma_start(wq_sb[:], wq.ap().rearrange("(c p) h -> p c h", p=128))
        wk_sb = singles.tile([128, DC, H], BF16)
        nc.sync.dma_start(wk_sb[:], wk.ap().rearrange("(c p) h -> p c h", p=128))
        qt_sb = singles.tile([128, DC, Q], BF16)
        nc.sync.dma_start(qt_sb[:], qT.ap().rearrange("(c p) q -> p c q", p=128))
        wv_sb = singles.tile([128, HT], BF16)
        nc.sync.dma_start(wv_sb[:], wv2.ap())
        ident = singles.tile([128, 128], BF16)
        make_identity(nc, ident[:])

        # ---- projections: qh [128, HT, Q] f32, kh [128, HT, KEXT] bf16 --
        # kT is streamed in 512-column chunks so the first S-build can
        # start long before the full 2.3MB load lands
        qh_sb = singles.tile([128, HT, Q], F32)
        kh_sb = singles.tile([128, HT, KEXT], BF16)
        for ht in range(HT):
            ps = psum.tile([128, 512], F32, tag="misc", bufs=2)
            for dc in range(DC):
                nc.tensor.matmul(
                    ps[:, :Q],
                    lhsT=wq_sb[:, dc, ht * 128 : (ht + 1) * 128],
                    rhs=qt_sb[:, dc, :],
                    start=(dc == 0),
                    stop=(dc == DC - 1),
                )
            nc.vector.tensor_copy(qh_sb[:, ht, :], ps[:, :Q])
        kt_r = kT.ap().rearrange("(c p) k -> p c k", p=128)
        for kc in range((KEXT + 511) // 512):
            cw = min(512, KEXT - kc * 512)
            ktc = work.tile([128, DC, 512], BF16, tag="ktc", bufs=3, name="ktc")
            nc.sync.dma_start(
                ktc[:, :, :cw], kt_r[:, :, kc * 512 : kc * 512 + cw]
            )
            for ht in range(HT):
                ps = psum.tile([128, 512], F32, tag="misc", bufs=2)
                for dc in range(DC):
                    nc.tensor.matmul(
                        ps[:, :cw],
                        lhsT=wk_sb[:, dc, ht * 128 : (ht + 1) * 128],
                        rhs=ktc[:, dc, :cw],
                        start=(dc == 0),
                        stop=(dc == DC - 1),
                    )
                nc.vector.tensor_copy(
                    kh_sb[:, ht, kc * 512 : kc * 512 + cw], ps[:, :cw]
                )

        # ---- scores + per-group epilogue --------------------------------
        av = psum.tile([128, 1024], F32, tag="av", bufs=1)
        slabs = [
            psum.tile([128, 1024], F32, tag=f"slab{i}", name=f"slab{i}")
            for i in range(2)
        ]
        for s in slabs:
            nc.vector.memset(s[:], 0.0)

        slab_i = 0
        kt_done = 0
        for g in range(B):
            kw, off = kws[g], int(koff[g])
            nchunk = (kw + 511) // 512
            scores_g = work.tile([128, kw], F32, tag="scores", name="scores_g")
            nc.vector.memset(scores_g[:], NEG)
            for jg in range(QPC // 8):  # two 8-row chunks per group
                s0 = work.tile([128, 8, kw], BF16, tag="S0", bufs=3, name="s0")
                s1 = work.tile([128, 8, kw], BF16, tag="S1", bufs=3, name="s1")
                for j in range(8):
                    slot = g * QPC + jg * 8 + j
                    nc.vector.tensor_scalar_add(
                        s0[:, j, :],
                        kh_sb[:, 0, off : off + kw],
                        qh_sb[:, 0, slot : slot + 1],
                    )
                    nc.vector.tensor_scalar_add(
                        s1[:, j, :],
                        kh_sb[:, 1, off : off + kw],
                        qh_sb[:, 1, slot : slot + 1],
                    )
                # tanh in place
                nc.scalar.activation(
                    s0[:], s0[:], mybir.ActivationFunctionType.Tanh
                )
                nc.scalar.activation(
                    s1[:], s1[:], mybir.ActivationFunctionType.Tanh
                )
                for half in range(2):  # 4-slot slabs
                    slab = slabs[slab_i % 2]
                    slab_i += 1
                    for j4 in range(4):
                        j = half * 4 + j4
                        for c in range(nchunk):
                            cw = min(512, kw - c * 512)
                            out_ap = slab[
                                32 * j4 : 32 * j4 + 1, c * 512 : c * 512 + cw
                            ]
                            nc.tensor.matmul(
                                out_ap,
                                lhsT=wv_sb[:, 0:1],
                                rhs=s0[:, j, c * 512 : c * 512 + cw],
                                start=True,
                                stop=False,
                                tile_position=(0, 32 * j4),
                            )
                            nc.tensor.matmul(
                                out_ap,
                                lhsT=wv_sb[:, 1:2],
                                rhs=s1[:, j, c * 512 : c * 512 + cw],
                                start=False,
                                stop=True,
                                tile_position=(0, 32 * j4),
                            )
                    stage = work.tile(
                        [128, 1024], F32, tag="stage", bufs=3, name="stage"
                    )
                    nc.vector.tensor_copy(stage[:, :kw], slab[:, :kw])
                    row0 = g * QPC + jg * 8 + half * 4
                    nc.gpsimd.dma_start(
                        scores_g[row0 : row0 + 4, :kw],
                        stage[0:128:32, :kw],
                    )

            # group epilogue: exp over this group's columns, transpose its
            # k-tiles, accumulate AV (runs while later groups compute)
            exp_g = work.tile([128, kw], BF16, tag="exp", name="exp_g")
            nc.scalar.activation(
                exp_g[:], scores_g[:], mybir.ActivationFunctionType.Exp
            )
            for t in range(kw // 128):
                kt = off // 128 + t
                vt = work.tile([128, VE], BF16, tag="vt", bufs=4, name="vt")
                nc.sync.dma_start(vt[:], vext[kt * 128 : (kt + 1) * 128, :])
                pt = psum.tile([128, 128], BF16, tag="misc", bufs=2)
                nc.tensor.transpose(
                    pt[:], exp_g[:, t * 128 : (t + 1) * 128], ident[:]
                )
                expt = work.tile([128, 128], BF16, tag="expT", bufs=4, name="expt")
                nc.vector.tensor_copy(expt[:], pt[:])
                nc.tensor.matmul(
                    av[:, 0:V],
                    lhsT=expt[:],
                    rhs=vt[:, 0:V],
                    start=(kt_done == 0),
                    stop=(kt_done == KT - 1),
                )
                nc.tensor.matmul(
                    av[:, 512:513],
                    lhsT=expt[:],
                    rhs=vt[:, V : V + 1],
                    start=(kt_done == 0),
                    stop=(kt_done == KT - 1),
                )
                kt_done += 1

        rl = singles.tile([128, 1], F32)
        nc.vector.reciprocal(rl[:], av[:, 512:513])
        out_sb = singles.tile([128, V], F32)
        nc.vector.tensor_scalar_mul(out_sb[:], av[:, 0:V], rl[:])
        nc.gpsimd.dma_start(out.ap(), out_sb[:])

    nc.compile()
    return nc


_CACHE = {}


def _install_profile_shim():
    """Provide antenv.axon_hooks (absent in this image) so
    run_bass_kernel_spmd(trace=True) can capture NTFF profiles through
    libaxon_pjrt.so, mirroring trn_agent_boot's bootstrap."""
    import types

    if "antenv.axon_hooks" not in sys.modules:
        mod = types.ModuleType("antenv.axon_hooks")
        state = {}
        mod.set_axon_ntff_profile_hook = lambda h: state.__setitem__("h", h)
        mod.get_axon_ntff_profile_hook = lambda: state.get("h")
        sys.modules["antenv.axon_hooks"] = mod
        import antenv

        antenv.axon_hooks = mod
        if "/root/.axon_site" not in sys.path:
            sys.path.insert(0, "/root/.axon_site")
        from trn_agent_boot.trn_boot import _ntff_profile_via_ctypes

        hook = _ntff_profile_via_ctypes("/opt/axon/libaxon_pjrt.so")
        mod.set_axon_ntff_profile_hook(hook)

        import concourse.bass_utils as bu

        orig_upload = bu.upload_artifacts

        def _safe_upload(tmpdir):
            try:
                return orig_upload(tmpdir)
            except Exception:
                return f"local:{tmpdir}"

        bu.upload_artifacts = _safe_upload


def _get_graph(valid_lens):
    key = tuple(int(v) for v in valid_lens)
    if _CACHE.get("key") != key:
        _CACHE["nc"] = _build_graph(valid_lens)
        _CACHE["key"] = key
    return _CACHE["nc"]


def _make_in_maps(queries, keys, values, valid_lens):
    kws, koff, KEXT = _kw_template(valid_lens)
    kT = np.zeros((D, KEXT), dtype=np.float32)
    vext = np.zeros((KEXT, VE), dtype=np.float32)
    for b in range(B):
        vl = int(valid_lens[b])
        kw, off = kws[b], int(koff[b])
        kT[:, off : off + kw] = keys[b, :kw].T
        vext[off : off + vl, :V] = values[b, :vl]
        vext[off : off + vl, V] = 1.0
    kT_bf = kT.astype(NP_BF16)
    vext_bf = vext.astype(NP_BF16)
    in_maps = []
    for c in range(B):
        qrows = np.concatenate(
            [queries[b, c * QPC : (c + 1) * QPC] for b in range(B)], axis=0
        )  # [128, D]; slot 16*b + r = (batch b, row 16*c + r)
        in_maps.append(
            {
                "qT": np.ascontiguousarray(qrows.T).astype(NP_BF16),
                "kT": kT_bf,
                "vext": vext_bf,
                "wq": _CACHE["wq_bf"],
                "wk": _CACHE["wk_bf"],
                "wv2": _CACHE["wv2_bf"],
            }
        )
    return in_maps


def kernel(
    queries, keys, values, valid_lens, Wq, Wk, wv, _profile=False, **_unused
):
    queries = np.asarray(queries, dtype=np.float32)
    keys = np.asarray(keys, dtype=np.float32)
    values = np.asarray(values, dtype=np.float32)
    valid_lens = np.asarray(valid_lens)
    _CACHE["wq_bf"] = np.asarray(Wq, np.float32).astype(NP_BF16)
    _CACHE["wk_bf"] = np.asarray(Wk, np.float32).astype(NP_BF16)
    _CACHE["wv2_bf"] = (
        np.asarray(wv, np.float32).reshape(HT, 128).T.copy().astype(NP_BF16)
    )

    nc = _get_graph(valid_lens)
    in_maps = _make_in_maps(queries, keys, values, valid_lens)
    kwargs = {}
    if _profile:
        _install_profile_shim()
        tdir = "/root/problem/trace_out"
        os.makedirs(tdir, exist_ok=True)
        kwargs["tmpdir"] = tdir
    res = run_bass_kernel_spmd(
        nc, in_maps, core_ids=list(range(B)), trace=_profile, **kwargs
    )
    out = np.zeros((B, Q, V), dtype=np.float32)
    for c in range(B):
        oc = np.asarray(res.results[c]["out"], dtype=np.float32)
        for b in range(B):
            out[b, c * QPC : (c + 1) * QPC] = oc[b * QPC : (b + 1) * QPC]
    if _profile:
        _CACHE["last_result"] = res
    return out


# revision 24
# speedup vs baseline: 1.5389x; 1.1636x over previous
"""Additive attention (Bahdanau) Trainium2 kernel, SPMD over 8 NeuronCores.

Math per batch b (see reference):
    q = queries[b] @ Wq                  [Q=128, H=256]
    k = keys[b]    @ Wk                  [K=1024, H=256]
    scores[i,j] = sum_h wv[h] * tanh(q[i,h] + k[j,h])
    attn = masked_softmax(scores, valid_len[b])
    out[b] = attn @ values[b]            [Q, V=512]

Sharding: sequence-parallel q-striping. Each core takes 16 q-rows of EVERY
batch and only the valid k-range of each batch (rounded up to 128). Per-core
work = sum_b 16*ceil(vl_b/128)*128 columns -- perfectly balanced for any
valid_lens, no collectives (softmax is per-q-row and stays core-local).

Device pipeline (per core), h-on-partitions layout:
  - qh = (queries_rows @ Wq).T  [2][128h, 128 q-slots]  (slot 16*b+r = batch
    b, row 16*core+r; host pre-transposes queries/keys so the contraction
    dim d is leading)
  - kh = (keys @ Wk).T          [2][128h, KEXT]  (KEXT = sum_b KW_b)
  - per q-slot: S = kh[:, group cols] + qh[:, slot]  (DVE tensor_scalar_add,
    bf16 4x mode), tanh in place (ACT), then scores row = wv.T @ feat via
    M=1 matmuls accumulated over the two h-tiles into PSUM slab rows at
    partitions {0,32,64,96}
  - slab -> stage (DVE copy) -> strided SBUF DMA -> scores_sb[q-slots, cols]
  - scores_sb pre-filled with -60 so cross-batch cells exp to ~0
  - exp (no max subtraction: |scores| <= sum|wv| ~ 13, safely in fp32 range)
  - mask is baked into values_ext: rows k >= vl zeroed, last column is
    1[k < vl], so attn @ values_ext also yields the softmax denominator
  - expT tiles via PE transpose; out = (expT.T @ values_ext) * recip(l)
"""

import os
import sys

import numpy as np

for _p in ("/opt/trn_rl_repo", "/root/.axon_site/_ro/trn_rl_repo"):
    if os.path.isdir(_p) and _p not in sys.path:
        sys.path.insert(0, _p)

os.environ.setdefault("MYCRO_LOCAL_CACHE", "1")

import ml_dtypes  # noqa: E402
from contextlib import ExitStack  # noqa: E402

import concourse.bass as bass  # noqa: E402
import concourse.tile as tile  # noqa: E402
from concourse import bacc, mybir  # noqa: E402
from concourse.bass_utils import run_bass_kernel_spmd  # noqa: E402
from concourse.masks import make_identity  # noqa: E402

BF16 = mybir.dt.bfloat16
F32 = mybir.dt.float32
NP_BF16 = ml_dtypes.bfloat16

B, Q, K, D, H, V = 8, 128, 1024, 512, 256, 512
DC = D // 128   # 4 contraction tiles for the projections
HT = H // 128   # 2 h-tiles
QPC = Q // B    # 16 q-rows per (batch, core)
VE = V + 1      # values extended with a ones column (softmax denominator)
NEG = -60.0     # filler for never-written score cells; exp(-60) ~ 9e-27


def _kw_template(valid_lens):
    """Group order: 2nd-smallest kw first (fast pipeline start), the rest
    descending, smallest last (short epilogue tail). Returns per-GROUP
    (batch index, kw, koff); everything (kT/v packing, q slots, scores
    columns) uses this order."""
    kwb = [max(128, int(-(-int(v) // 128) * 128)) for v in valid_lens]
    by_kw = sorted(range(len(kwb)), key=lambda b: (kwb[b], b))
    order = [by_kw[1]] + sorted(by_kw[2:], key=lambda b: (-kwb[b], b)) + [by_kw[0]]
    kw = [kwb[b] for b in order]
    koff = np.concatenate([[0], np.cumsum(kw)]).astype(int)
    return order, kw, koff, int(koff[-1])


def _build_graph(valid_lens):
    _order, kws, koff, KEXT = _kw_template(valid_lens)
    nc = bacc.Bacc(
        "TRN2",
        target_bir_lowering=False,
        debug=False,
        num_devices=8,
    )

    qT = nc.dram_tensor("qT", [D, Q], BF16, kind="ExternalInput")
    kT = nc.dram_tensor("kT", [D, KEXT], BF16, kind="ExternalInput")
    vext = nc.dram_tensor("vext", [KEXT, VE], BF16, kind="ExternalInput")
    wq = nc.dram_tensor("wq", [D, H], BF16, kind="ExternalInput")
    wk = nc.dram_tensor("wk", [D, H], BF16, kind="ExternalInput")
    wv2 = nc.dram_tensor("wv2", [128, HT], BF16, kind="ExternalInput")
    out = nc.dram_tensor("out", [Q, V], F32, kind="ExternalOutput")

    KT = KEXT // 128  # 128-col k-tiles (template is 128-aligned)

    with tile.TileContext(nc) as tc, ExitStack() as ctx:
        singles = ctx.enter_context(tc.tile_pool(name="singles", bufs=1))
        work = ctx.enter_context(tc.tile_pool(name="work", bufs=2))
        psum = ctx.enter_context(tc.tile_pool(name="psum", bufs=1, space="PSUM"))

        # ---- load inputs ------------------------------------------------
        # tiny warmup activation so the ~2.7us ACT table load overlaps DMAs
        warm = singles.tile([1, 2], F32)
        nc.vector.memset(warm[:], 0.0)
        nc.scalar.activation(warm[:], warm[:], mybir.ActivationFunctionType.Tanh)

        wq_sb = singles.tile([128, DC, H], BF16)
        nc.sync.dma_start(wq_sb[:], wq.ap().rearrange("(c p) h -> p c h", p=128))
        wk_sb = singles.tile([128, DC, H], BF16)
        nc.sync.dma_start(wk_sb[:], wk.ap().rearrange("(c p) h -> p c h", p=128))
        qt_sb = singles.tile([128, DC, Q], BF16)
        nc.sync.dma_start(qt_sb[:], qT.ap().rearrange("(c p) q -> p c q", p=128))
        wv_sb = singles.tile([128, HT], BF16)
        nc.sync.dma_start(wv_sb[:], wv2.ap())
        ident = singles.tile([128, 128], BF16)
        make_identity(nc, ident[:])

        # ---- projections: qh [128, HT, Q] f32, kh [128, HT, KEXT] bf16 --
        # kT is streamed in 512-column chunks so the first S-build can
        # start long before the full 2.3MB load lands
        qh_sb = singles.tile([128, HT, Q], F32)
        kh_sb = singles.tile([128, HT, KEXT], BF16)
        for ht in range(HT):
            ps = psum.tile([128, 512], F32, tag="misc", bufs=2)
            for dc in range(DC):
                nc.tensor.matmul(
                    ps[:, :Q],
                    lhsT=wq_sb[:, dc, ht * 128 : (ht + 1) * 128],
                    rhs=qt_sb[:, dc, :],
                    start=(dc == 0),
                    stop=(dc == DC - 1),
                )
            nc.vector.tensor_copy(qh_sb[:, ht, :], ps[:, :Q])
        kt_r = kT.ap().rearrange("(c p) k -> p c k", p=128)
        for kc in range((KEXT + 511) // 512):
            cw = min(512, KEXT - kc * 512)
            ktc = work.tile([128, DC, 512], BF16, tag="ktc", bufs=3, name="ktc")
            nc.sync.dma_start(
                ktc[:, :, :cw], kt_r[:, :, kc * 512 : kc * 512 + cw]
            )
            for ht in range(HT):
                ps = psum.tile([128, 512], F32, tag="misc", bufs=2)
                for dc in range(DC):
                    nc.tensor.matmul(
                        ps[:, :cw],
                        lhsT=wk_sb[:, dc, ht * 128 : (ht + 1) * 128],
                        rhs=ktc[:, dc, :cw],
                        start=(dc == 0),
                        stop=(dc == DC - 1),
                    )
                nc.vector.tensor_copy(
                    kh_sb[:, ht, kc * 512 : kc * 512 + cw], ps[:, :cw]
                )

        # ---- scores + per-group epilogue --------------------------------
        av = psum.tile([128, 1024], F32, tag="av", bufs=1)
        slabs = [
            psum.tile([128, 1024], F32, tag=f"slab{i}", name=f"slab{i}")
            for i in range(2)
        ]
        for s in slabs:
            nc.vector.memset(s[:], 0.0)

        slab_i = 0
        kt_done = 0
        for g in range(B):
            kw, off = kws[g], int(koff[g])
            nchunk = (kw + 511) // 512
            scores_g = work.tile([128, kw], F32, tag="scores", name="scores_g")
            nc.vector.memset(scores_g[:], NEG)
            for jg in range(QPC // 8):  # two 8-row chunks per group
                s0 = work.tile([128, 8, kw], BF16, tag="S0", bufs=3, name="s0")
                s1 = work.tile([128, 8, kw], BF16, tag="S1", bufs=3, name="s1")
                for ht, st in ((0, s0), (1, s1)):
                    for j in range(8):
                        slot = g * QPC + jg * 8 + j
                        nc.vector.tensor_scalar_add(
                            st[:, j, :],
                            kh_sb[:, ht, off : off + kw],
                            qh_sb[:, ht, slot : slot + 1],
                        )
                # tanh in place
                nc.scalar.activation(
                    s0[:], s0[:], mybir.ActivationFunctionType.Tanh
                )
                nc.scalar.activation(
                    s1[:], s1[:], mybir.ActivationFunctionType.Tanh
                )
                for half in range(2):  # 4-slot slabs
                    slab = slabs[slab_i % 2]
                    slab_i += 1
                    for j4 in range(4):
                        j = half * 4 + j4
                        for c in range(nchunk):
                            cw = min(512, kw - c * 512)
                            out_ap = slab[
                                32 * j4 : 32 * j4 + 1, c * 512 : c * 512 + cw
                            ]
                            nc.tensor.matmul(
                                out_ap,
                                lhsT=wv_sb[:, 0:1],
                                rhs=s0[:, j, c * 512 : c * 512 + cw],
                                start=True,
                                stop=False,
                                tile_position=(0, 32 * j4),
                            )
                            nc.tensor.matmul(
                                out_ap,
                                lhsT=wv_sb[:, 1:2],
                                rhs=s1[:, j, c * 512 : c * 512 + cw],
                                start=False,
                                stop=True,
                                tile_position=(0, 32 * j4),
                            )
                    stage = work.tile(
                        [128, 1024], F32, tag="stage", bufs=3, name="stage"
                    )
                    nc.vector.tensor_copy(stage[:, :kw], slab[:, :kw])
                    row0 = g * QPC + jg * 8 + half * 4
                    nc.sync.dma_start(
                        scores_g[row0 : row0 + 4, :kw],
                        stage[0:128:32, :kw],
                    )

            # group epilogue: exp over this group's columns, transpose its
            # k-tiles, accumulate AV (runs while later groups compute)
            exp_g = work.tile([128, kw], BF16, tag="exp", name="exp_g")
            nc.scalar.activation(
                exp_g[:], scores_g[:], mybir.ActivationFunctionType.Exp
            )
            for t in range(kw // 128):
                kt = off // 128 + t
                vt = work.tile([128, VE], BF16, tag="vt", bufs=4, name="vt")
                nc.sync.dma_start(vt[:], vext[kt * 128 : (kt + 1) * 128, :])
                pt = psum.tile([128, 128], BF16, tag="misc", bufs=2)
                nc.tensor.transpose(
                    pt[:], exp_g[:, t * 128 : (t + 1) * 128], ident[:]
                )
                expt = work.tile([128, 128], BF16, tag="expT", bufs=4, name="expt")
                nc.vector.tensor_copy(expt[:], pt[:])
                nc.tensor.matmul(
                    av[:, 0:V],
                    lhsT=expt[:],
                    rhs=vt[:, 0:V],
                    start=(kt_done == 0),
                    stop=(kt_done == KT - 1),
                )
                nc.tensor.matmul(
                    av[:, 512:513],
                    lhsT=expt[:],
                    rhs=vt[:, V : V + 1],
                    start=(kt_done == 0),
                    stop=(kt_done == KT - 1),
                )
                kt_done += 1

        rl = singles.tile([128, 1], F32)
        nc.vector.reciprocal(rl[:], av[:, 512:513])
        out_sb = singles.tile([128, V], F32)
        nc.vector.tensor_scalar_mul(out_sb[:], av[:, 0:V], rl[:])
        nc.sync.dma_start(out.ap(), out_sb[:])

    nc.compile()
    return nc


_CACHE = {}


def _install_profile_shim():
    """Provide antenv.axon_hooks (absent in this image) so
    run_bass_kernel_spmd(trace=True) can capture NTFF profiles through
    libaxon_pjrt.so, mirroring trn_agent_boot's bootstrap."""
    import types

    if "antenv.axon_hooks" not in sys.modules:
        mod = types.ModuleType("antenv.axon_hooks")
        state = {}
        mod.set_axon_ntff_profile_hook = lambda h: state.__setitem__("h", h)
        mod.get_axon_ntff_profile_hook = lambda: state.get("h")
        sys.modules["antenv.axon_hooks"] = mod
        import antenv

        antenv.axon_hooks = mod
        if "/root/.axon_site" not in sys.path:
            sys.path.insert(0, "/root/.axon_site")
        from trn_agent_boot.trn_boot import _ntff_profile_via_ctypes

        hook = _ntff_profile_via_ctypes("/opt/axon/libaxon_pjrt.so")
        mod.set_axon_ntff_profile_hook(hook)

        import concourse.bass_utils as bu

        orig_upload = bu.upload_artifacts

        def _safe_upload(tmpdir):
            try:
                return orig_upload(tmpdir)
            except Exception:
                return f"local:{tmpdir}"

        bu.upload_artifacts = _safe_upload


def _get_graph(valid_lens):
    key = tuple(int(v) for v in valid_lens)
    if _CACHE.get("key") != key:
        _CACHE["nc"] = _build_graph(valid_lens)
        _CACHE["key"] = key
    return _CACHE["nc"]


def _make_in_maps(queries, keys, values, valid_lens):
    order, kws, koff, KEXT = _kw_template(valid_lens)
    kT = np.zeros((D, KEXT), dtype=np.float32)
    vext = np.zeros((KEXT, VE), dtype=np.float32)
    for g, b in enumerate(order):
        vl = int(valid_lens[b])
        kw, off = kws[g], int(koff[g])
        kT[:, off : off + kw] = keys[b, :kw].T
        vext[off : off + vl, :V] = values[b, :vl]
        vext[off : off + vl, V] = 1.0
    kT_bf = kT.astype(NP_BF16)
    vext_bf = vext.astype(NP_BF16)
    in_maps = []
    for c in range(B):
        qrows = np.concatenate(
            [queries[b, c * QPC : (c + 1) * QPC] for b in order], axis=0
        )  # [128, D]; slot 16*g + r = (batch order[g], row 16*c + r)
        in_maps.append(
            {
                "qT": np.ascontiguousarray(qrows.T).astype(NP_BF16),
                "kT": kT_bf,
                "vext": vext_bf,
                "wq": _CACHE["wq_bf"],
                "wk": _CACHE["wk_bf"],
                "wv2": _CACHE["wv2_bf"],
            }
        )
    return in_maps


def kernel(
    queries, keys, values, valid_lens, Wq, Wk, wv, _profile=False, **_unused
):
    queries = np.asarray(queries, dtype=np.float32)
    keys = np.asarray(keys, dtype=np.float32)
    values = np.asarray(values, dtype=np.float32)
    valid_lens = np.asarray(valid_lens)
    _CACHE["wq_bf"] = np.asarray(Wq, np.float32).astype(NP_BF16)
    _CACHE["wk_bf"] = np.asarray(Wk, np.float32).astype(NP_BF16)
    _CACHE["wv2_bf"] = (
        np.asarray(wv, np.float32).reshape(HT, 128).T.copy().astype(NP_BF16)
    )

    nc = _get_graph(valid_lens)
    in_maps = _make_in_maps(queries, keys, values, valid_lens)
    kwargs = {}
    if _profile:
        _install_profile_shim()
        tdir = "/root/problem/trace_out"
        os.makedirs(tdir, exist_ok=True)
        kwargs["tmpdir"] = tdir
    res = run_bass_kernel_spmd(
        nc, in_maps, core_ids=list(range(B)), trace=_profile, **kwargs
    )
    order = _kw_template(valid_lens)[0]
    out = np.zeros((B, Q, V), dtype=np.float32)
    for c in range(B):
        oc = np.asarray(res.results[c]["out"], dtype=np.float32)
        for g, b in enumerate(order):
            out[b, c * QPC : (c + 1) * QPC] = oc[g * QPC : (g + 1) * QPC]
    if _profile:
        _CACHE["last_result"] = res
    return out
